# revision 17
# baseline (speedup 1.0000x reference)
"""Trainium2 Bass kernel for BCNLayer (3x3 per-position-weighted spatial
shift conv over a 128x128 grid + sigmoid).

y[yo,xo,b] = sigmoid( sum_{dy,dx in {-1,0,1}} w[dy+1,dx+1,(yo-dy)*128+(xo-dx)]
                      * x[(yo-dy)*128+(xo-dx), b] )   (zero outside the grid)

Formulation: for each output row yo, y_row[yo] = sigmoid( sum_{yi in
{yo-1,yo,yo+1}} T[dy,yi].T @ x_row[yi] ) where T[dy,yi] is a 128x128
tridiagonal matrix holding the three dx weight vectors of input row yi on
its diagonals (dy = yo-yi).  T matrices are built on-chip in f32 from an
SBUF weight image: one gpsimd affine_select zero-fills the block and
places the j=0 diagonal (per-partition iota c - xi == 0 against a
broadcast weight read), then two DVE copy_predicated ops add the j=1,2
diagonals.  A 130-wide buffer with the matmul reading cols 1:129 makes
the x-boundary masking fall out of the padding columns.

Matmuls run float32r x float32r (1 cycle/row on the PE; walrus rejects
mixing 32-bit with 16-bit operands).  float32r is bit-identical to f32
in memory -- the lhsT reads the f32 T tiles through a bitcast -- and x is DMAed
unchanged by HWDGE (SP ring) into tiles declared f32r -- no SWDGE cast,
which keeps the Pool engine free of descriptor-generation work.  Output
is computed by the ACT sigmoid into fp16 and stored on the ACT HWDGE
ring (sigmoid in [0,1]: fp16 adds <= ~2.4e-4 abs error; host upcasts).

Sharding: data-parallel over batch, 4096/8 = 512 columns per core.
"""

import numpy as np

H = 128
W = 128
HW = H * W
B = 4096
NCORES = 8
BC = B // NCORES  # 512 batch columns per core
G = 8  # yi rows per weight-group tile
R = 4  # y rows per store DMA (4 * 128 part * 512 * 2B = 512 KiB fp16)
LR = 8  # x rows per load DMA (8 * 128 part * 512 * 4B = 2 MiB)

OUT_FP8 = True

_CACHE = {}


def _make_tile_context_cls():
    import concourse.tile as tile
    import bass_rust

    class SplitDrainTileContext(tile.TileContext):
        """The walrus build in this container accepts at most one sem-wait
        per instruction; Tile freely emits several (e.g. a matmul waiting
        on both operand DMA lanes).  Split the extras onto single-wait
        nops emitted just before the instruction on the same engine."""

        def _add_instruction(self, inst):
            from concourse import mybir as _mybir

            si = inst.sync_info
            if si is not None and si.on_wait and len(si.on_wait) > 1:
                waits = list(si.on_wait)
                si.on_wait = [waits[-1]]
                for w in waits[:-1]:
                    nop = _mybir.InstNoOp(
                        name=self.nc.get_next_instruction_name(),
                        ins=[],
                        outs=[],
                    )
                    nop.engine = inst.engine
                    nop.sync_info = _mybir.SyncInfo(on_wait=[w], on_update=[])
                    super()._add_instruction(nop)
            super()._add_instruction(inst)

        def _drain_and_barrier(self, tick_clock, wait_clock):
            collector = self.nc.sync.nop(nofuse=True, hint="tail_waits")
            wait_clock.add_sem_waits(
                collector.ins,
                bass_rust.ScopedClock({None: tick_clock.global_clock}),
            )
            si = collector.ins.sync_info
            waits = list(si.on_wait) if si is not None and si.on_wait else []
            if len(waits) > 1:
                si.on_wait = [waits[0]]
                from concourse import mybir as _mybir

                for w in waits[1:]:
                    n = self.nc.sync.nop(nofuse=True, hint="tail_waits")
                    n.ins.sync_info = _mybir.SyncInfo(on_wait=[w], on_update=[])
            self.nc.sync.drain()
            self.nc.all_engine_barrier()
            assert self.sems is not None
            popped = self.nc._tile_sem_poison_stack.pop()
            assert popped is self._sem_poison
            self.nc.clear_and_free_semaphores(
                list(self.sems.allocated().values())
            )
            self.nc.all_engine_barrier()

    return SplitDrainTileContext


def _build_nc(repeat=1):
    import concourse.bass as bass
    import concourse.tile as tile
    import concourse.mybir as mybir
    from concourse.ap import AP

    tile_context_cls = _make_tile_context_cls()
    f32 = mybir.dt.float32
    f32r = mybir.dt.float32r
    bf16 = mybir.dt.bfloat16
    f16 = mybir.dt.float16
    i16 = mybir.dt.int16
    nc = bass.Bass("TRN2", target_bir_lowering=False, debug=False)
    # x declared float32r: bit-identical to the f32 bytes in DRAM, lets
    # HWDGE move it without a dtype cast while the PE reads the tiles on
    # its 1-cycle/row f32r path.
    x = nc.dram_tensor("x", [HW, BC], f32r, kind="ExternalInput")
    # wsb[xi, (i, yi, j)] = w[i, j, yi*128+xi]: the per-partition SBUF
    # image of the weights, prepared host-side in bf16
    wsb_d = nc.dram_tensor("wsb", [128, 1152], f32, kind="ExternalInput")
    # y stored as t = tanh(z/2) in fp8-e4m3: sigmoid(z) = (1+t)/2 is
    # recovered on the host, halving the e4m3 quantization error to
    # <= 2^-6/4 + tanh-table error (~1.7e-2 worst case, within the 2e-2
    # budget) while halving the output DMA traffic vs fp16
    f8 = mybir.dt.float8e4
    y = nc.dram_tensor("y", [HW, BC], f8 if OUT_FP8 else f16,
                       kind="ExternalOutput")

    NCH = H // LR  # 16 x row-chunks
    NGR = H // G  # 16 weight groups
    TW = 130  # T used width: col c = xi + j, lhsT reads cols 1:129
    TWS = 131  # T stored stride (!=TW so strided APs never dim-merge)

    with tile_context_cls(nc) as tc:
        with (
            tc.tile_pool(name="cn", bufs=1) as cpool,
            tc.tile_pool(name="xp", bufs=5) as xpool,
            tc.tile_pool(name="rp", bufs=3) as rpool,
            tc.tile_pool(name="tp", bufs=5) as tpool,
            tc.tile_pool(name="op", bufs=4) as opool,
            tc.tile_pool(name="ps", bufs=4, space="PSUM") as ppool,
        ):
            # one-time: weight image + one-hot diagonal masks for j=1,2
            wsb = cpool.tile([128, 1152], f32)
            nc.scalar.dma_start(out=wsb[:], in_=wsb_d.ap())
            ones = cpool.tile([128, TW], i16)
            nc.gpsimd.memset(ones[:], 1)
            masks = cpool.tile([128, 3, TW], i16)
            for j in range(1, 3):
                # D_j[xi, c] = 1 where c - xi - j == 0
                nc.gpsimd.affine_select(
                    masks[:, j, :], ones[:],
                    pattern=[[1, TW]], base=-j, channel_multiplier=-1,
                    compare_op=mybir.AluOpType.is_equal, fill=0,
                )

            xt = {}
            tt = {}

            def load_chunk(c):  # noqa: closure rebound per repeat
                if c in xt or c >= NCH:
                    return
                t = xpool.tile([128, LR, BC], f32r, tag="xchunk")
                # x rows [c*LR*128, (c+1)*LR*128): one 2 MiB HWDGE DMA on
                # the SP ring; descriptor generation happens in RTL so the
                # Pool engine stays free for T-building.
                src = AP(
                    x.ap().tensor,
                    c * LR * 128 * BC,
                    [[BC, 128], [128 * BC, LR], [1, BC]],
                )
                nc.sync.dma_start(out=t[:], in_=src)
                xt[c] = t

            def load_group(g):
                if g in tt or g >= NGR:
                    return
                traw = rpool.tile([128, 3, G, TWS], f32, tag="Traw")
                t = tpool.tile([128, 3, G, TWS], f32r, tag="T")
                ta = traw[:]
                ti = t[:]
                wv = wsb[:]
                for i in range(3):
                    out_i = AP(ta.tensor, ta.offset + i * G * TWS,
                               [[3 * G * TWS, 128], [TWS, G], [1, TWS]])
                    out_w = AP(ta.tensor, ta.offset + i * G * TWS,
                               [[3 * G * TWS, 128], [TWS, G], [1, TW]])

                    def wb(j, width):
                        return AP(wv.tensor,
                                  wv.offset + i * 384 + g * G * 3 + j,
                                  [[1152, 128], [3, G], [0, width]])

                    # j=0 via affine_select: zero-fills the block (incl.
                    # the pad column) and places the diagonal (c - xi ==
                    # 0) in one gpsimd pass
                    nc.gpsimd.affine_select(
                        out_i, wb(0, TWS),
                        pattern=[[0, G], [1, TWS]], base=0,
                        channel_multiplier=-1,
                        compare_op=mybir.AluOpType.is_equal, fill=0.0,
                    )
                    for j in range(1, 3):
                        ma = masks[:, j, :]
                        mb = AP(ma.tensor, ma.offset,
                                [[3 * TW, 128], [0, G], [1, TW]])
                        nc.vector.copy_predicated(out_w, mb, wb(j, TW))
                    # round to f32r (the BIR verifier requires every
                    # writer of matmul operand bytes to produce f32r, so
                    # the rounded copy lives in its own tile); flat 2D
                    # copy, alternating ACT / Pool so neither queue's
                    # in-order head-of-line blocking stalls the sigmoids
                    # (ACT) or the diagonal placements (Pool)
                    flat_in = AP(ta.tensor, ta.offset + i * G * TWS,
                                 [[3 * G * TWS, 128], [1, G * TWS]])
                    flat_out = AP(ti.tensor, ti.offset + i * G * TWS,
                                  [[3 * G * TWS, 128], [1, G * TWS]])
                    if (3 * g + i) % 2 == 0:
                        nc.scalar.activation(
                            flat_out, flat_in,
                            mybir.ActivationFunctionType.Copy,
                        )
                    else:
                        nc.gpsimd.tensor_scalar_mul(flat_out, flat_in, 1.0)
                tt[g] = t

            rep_range = range(repeat)
            for _rep in rep_range:
              if _rep:
                xt.clear()
                tt.clear()
              # prime the pipeline: fill every buffer slot so the DMA
              # engines saturate from t=0 instead of ramping with the loop
              for _c in range(4):
                  load_chunk(_c)
              for _g in range(5):
                  load_group(_g)

              ystage = None
              for yp in range(H // 2):
                  # prefetch beyond what this row-pair touches
                  load_chunk((2 * yp + 2) // LR + 1)
                  load_chunk((2 * yp + 2) // LR + 2)
                  load_group((2 * yp + 2) // G + 1)
                  load_group((2 * yp + 2) // G + 2)

                  # two output rows share a 2-bank PSUM tile so the
                  # sigmoid/tanh runs once per pair (halves the ACT op
                  # count and its PSUM-access fixed costs); each matmul
                  # still writes a single bank
                  pt = ppool.tile([128, 2 * BC], f32, tag="psum")
                  for half in range(2):
                      yo = 2 * yp + half
                      yis = [yi for yi in (yo - 1, yo, yo + 1)
                             if 0 <= yi < H]
                      for k, yi in enumerate(yis):
                          i_dy = yo - yi + 1
                          lhsT = tt[yi // G][:, i_dy, yi % G, 1 : 1 + 128]
                          rhs = xt[yi // LR][:, yi % LR, :]
                          nc.tensor.matmul(
                              pt[:, half * BC : (half + 1) * BC],
                              lhsT,
                              rhs,
                              start=(k == 0),
                              stop=(k == len(yis) - 1),
                          )

                  yo = 2 * yp
                  if yo % R == 0:
                      ystage = opool.tile([128, R, BC],
                                          f8 if OUT_FP8 else f16, tag="yst")
                  po = ystage[:]
                  pair_out = AP(po.tensor, po.offset + (yo % R) * BC,
                                [[R * BC, 128], [1, 2 * BC]])
                  if OUT_FP8:
                      nc.scalar.activation(
                          pair_out,
                          pt[:],
                          mybir.ActivationFunctionType.Tanh,
                          scale=0.5,
                      )
                  else:
                      nc.scalar.activation(
                          pair_out,
                          pt[:],
                          mybir.ActivationFunctionType.Sigmoid,
                      )
                  if yo % R == R - 2:
                      c = yo // R
                      dst = AP(
                          y.ap().tensor,
                          c * R * 128 * BC,
                          [[BC, 128], [128 * BC, R], [1, BC]],
                      )
                      # stores on the ACT-issued HWDGE ring so they never
                      # head-of-line-block the loads on the SP ring
                      nc.scalar.dma_start(out=dst, in_=ystage[:])
    return nc


def get_nc():
    if "nc" not in _CACHE:
        _CACHE["nc"] = _build_nc()
    return _CACHE["nc"]


def _prep_wsb(w: np.ndarray) -> np.ndarray:
    return np.ascontiguousarray(
        np.asarray(w, dtype=np.float32)
        .reshape(3, 3, H, W)
        .transpose(3, 0, 2, 1)
        .reshape(128, 1152)
    )


def kernel(x: np.ndarray, w: np.ndarray) -> np.ndarray:
    import time as _time

    from concourse.bass_utils import run_bass_kernel_spmd

    x = np.ascontiguousarray(x, dtype=np.float32)
    wsb = _prep_wsb(w)
    nc = get_nc()
    in_maps = [
        {"x": x[:, i * BC : (i + 1) * BC], "wsb": wsb} for i in range(NCORES)
    ]
    # The compile hook / remote execution path occasionally fails
    # transiently (observed: a flaky walrus invocation and a recoverable
    # NRT exec error); retry a few times before giving up.
    last_exc = None
    for attempt in range(4):
        try:
            res = run_bass_kernel_spmd(
                nc, in_maps, list(range(NCORES))
            ).results
            break
        except Exception as exc:  # noqa: BLE001
            last_exc = exc
            _time.sleep(2.0 * (attempt + 1))
    else:
        raise last_exc
    yfull = np.concatenate(
        [np.asarray(res[i]["y"]).astype(np.float32) for i in range(NCORES)],
        axis=1,
    )
    if OUT_FP8:
        # y stored as tanh(z/2): sigmoid(z) = (1 + tanh(z/2)) / 2
        yfull = 0.5 + 0.5 * yfull
    return np.ascontiguousarray(yfull, dtype=np.float32)


# revision 18
# speedup vs baseline: 2.3933x; 2.3933x over previous
"""Trainium2 Bass kernel for BCNLayer (3x3 per-position-weighted spatial
shift conv over a 128x128 grid + sigmoid).

y[yo,xo,b] = sigmoid( sum_{dy,dx in {-1,0,1}} w[dy+1,dx+1,(yo-dy)*128+(xo-dx)]
                      * x[(yo-dy)*128+(xo-dx), b] )   (zero outside the grid)

Formulation: for each output row yo, y_row[yo] = sigmoid( sum_{yi in
{yo-1,yo,yo+1}} T[dy,yi].T @ x_row[yi] ) where T[dy,yi] is a 128x128
tridiagonal matrix holding the three dx weight vectors of input row yi on
its diagonals (dy = yo-yi).  T matrices are built on-chip in f32 from an
SBUF weight image: one gpsimd affine_select zero-fills the block and
places the j=0 diagonal (per-partition iota c - xi == 0 against a
broadcast weight read), then two DVE copy_predicated ops add the j=1,2
diagonals.  A 130-wide buffer with the matmul reading cols 1:129 makes
the x-boundary masking fall out of the padding columns.

Matmuls run float32r x float32r (1 cycle/row on the PE; walrus rejects
mixing 32-bit with 16-bit operands).  float32r is bit-identical to f32
in memory -- the lhsT reads the f32 T tiles through a bitcast -- and x is DMAed
unchanged by HWDGE (SP ring) into tiles declared f32r -- no SWDGE cast,
which keeps the Pool engine free of descriptor-generation work.  Output
is computed by the ACT sigmoid into fp16 and stored on the ACT HWDGE
ring (sigmoid in [0,1]: fp16 adds <= ~2.4e-4 abs error; host upcasts).

Sharding: data-parallel over batch, 4096/8 = 512 columns per core.
"""

import numpy as np

H = 128
W = 128
HW = H * W
B = 4096
NCORES = 8
BC = B // NCORES  # 512 batch columns per core
G = 8  # yi rows per weight-group tile
R = 4  # y rows per store DMA (4 * 128 part * 512 * 2B = 512 KiB fp16)
LR = 8  # x rows per load DMA (8 * 128 part * 512 * 4B = 2 MiB)

OUT_FP8 = True

_CACHE = {}


def _make_tile_context_cls():
    import concourse.tile as tile
    import bass_rust

    class SplitDrainTileContext(tile.TileContext):
        """The walrus build in this container accepts at most one sem-wait
        per instruction; Tile freely emits several (e.g. a matmul waiting
        on both operand DMA lanes).  Split the extras onto single-wait
        nops emitted just before the instruction on the same engine."""

        def _add_instruction(self, inst):
            from concourse import mybir as _mybir

            si = inst.sync_info
            if si is not None and si.on_wait and len(si.on_wait) > 1:
                waits = list(si.on_wait)
                si.on_wait = [waits[-1]]
                for w in waits[:-1]:
                    nop = _mybir.InstNoOp(
                        name=self.nc.get_next_instruction_name(),
                        ins=[],
                        outs=[],
                    )
                    nop.engine = inst.engine
                    nop.sync_info = _mybir.SyncInfo(on_wait=[w], on_update=[])
                    super()._add_instruction(nop)
            super()._add_instruction(inst)

        def _drain_and_barrier(self, tick_clock, wait_clock):
            collector = self.nc.sync.nop(nofuse=True, hint="tail_waits")
            wait_clock.add_sem_waits(
                collector.ins,
                bass_rust.ScopedClock({None: tick_clock.global_clock}),
            )
            si = collector.ins.sync_info
            waits = list(si.on_wait) if si is not None and si.on_wait else []
            if len(waits) > 1:
                si.on_wait = [waits[0]]
                from concourse import mybir as _mybir

                for w in waits[1:]:
                    n = self.nc.sync.nop(nofuse=True, hint="tail_waits")
                    n.ins.sync_info = _mybir.SyncInfo(on_wait=[w], on_update=[])
            self.nc.sync.drain()
            self.nc.all_engine_barrier()
            assert self.sems is not None
            popped = self.nc._tile_sem_poison_stack.pop()
            assert popped is self._sem_poison
            self.nc.clear_and_free_semaphores(
                list(self.sems.allocated().values())
            )
            self.nc.all_engine_barrier()

    return SplitDrainTileContext


def _build_nc(repeat=1):
    import concourse.bass as bass
    import concourse.tile as tile
    import concourse.mybir as mybir
    from concourse.ap import AP

    tile_context_cls = _make_tile_context_cls()
    f32 = mybir.dt.float32
    f32r = mybir.dt.float32r
    bf16 = mybir.dt.bfloat16
    f16 = mybir.dt.float16
    i16 = mybir.dt.int16
    nc = bass.Bass("TRN2", target_bir_lowering=False, debug=False)
    # x declared float32r: bit-identical to the f32 bytes in DRAM, lets
    # HWDGE move it without a dtype cast while the PE reads the tiles on
    # its 1-cycle/row f32r path.
    x = nc.dram_tensor("x", [HW, BC], f32r, kind="ExternalInput")
    # wsb[xi, (i, yi, j)] = w[i, j, yi*128+xi]: the per-partition SBUF
    # image of the weights, prepared host-side in bf16
    wsb_d = nc.dram_tensor("wsb", [128, 1152], f32, kind="ExternalInput")
    # y stored as t = tanh(z/2) in fp8-e4m3: sigmoid(z) = (1+t)/2 is
    # recovered on the host, halving the e4m3 quantization error to
    # <= 2^-6/4 + tanh-table error (~1.7e-2 worst case, within the 2e-2
    # budget) while halving the output DMA traffic vs fp16
    f8 = mybir.dt.float8e4
    y = nc.dram_tensor("y", [HW, BC], f8 if OUT_FP8 else f16,
                       kind="ExternalOutput")

    NCH = H // LR  # 16 x row-chunks
    NGR = H // G  # 16 weight groups
    TW = 130  # T used width: col c = xi + j, lhsT reads cols 1:129
    TWS = 131  # T stored stride (!=TW so strided APs never dim-merge)

    with tile_context_cls(nc) as tc:
        with (
            tc.tile_pool(name="cn", bufs=1) as cpool,
            tc.tile_pool(name="xp", bufs=5) as xpool,
            tc.tile_pool(name="rp", bufs=3) as rpool,
            tc.tile_pool(name="tp", bufs=5) as tpool,
            tc.tile_pool(name="op", bufs=4) as opool,
            tc.tile_pool(name="ps", bufs=4, space="PSUM") as ppool,
        ):
            # one-time: weight image + one-hot diagonal masks for j=1,2
            wsb = cpool.tile([128, 1152], f32)
            nc.scalar.dma_start(out=wsb[:], in_=wsb_d.ap())
            ones = cpool.tile([128, TW], i16)
            nc.gpsimd.memset(ones[:], 1)
            masks = cpool.tile([128, 3, TW], i16)
            for j in range(1, 3):
                # D_j[xi, c] = 1 where c - xi - j == 0
                nc.gpsimd.affine_select(
                    masks[:, j, :], ones[:],
                    pattern=[[1, TW]], base=-j, channel_multiplier=-1,
                    compare_op=mybir.AluOpType.is_equal, fill=0,
                )

            xt = {}
            tt = {}

            def load_chunk(c):  # noqa: closure rebound per repeat
                if c in xt or c >= NCH:
                    return
                t = xpool.tile([128, LR, BC], f32r, tag="xchunk")
                # x rows [c*LR*128, (c+1)*LR*128): one 2 MiB HWDGE DMA on
                # the SP ring; descriptor generation happens in RTL so the
                # Pool engine stays free for T-building.
                src = AP(
                    x.ap().tensor,
                    c * LR * 128 * BC,
                    [[BC, 128], [128 * BC, LR], [1, BC]],
                )
                nc.sync.dma_start(out=t[:], in_=src)
                xt[c] = t

            def load_group(g):
                if g in tt or g >= NGR:
                    return
                traw = rpool.tile([128, 3, G, TWS], f32, tag="Traw")
                t = tpool.tile([128, 3, G, TWS], f32r, tag="T")
                ta = traw[:]
                ti = t[:]
                wv = wsb[:]
                for i in range(3):
                    out_i = AP(ta.tensor, ta.offset + i * G * TWS,
                               [[3 * G * TWS, 128], [TWS, G], [1, TWS]])
                    out_w = AP(ta.tensor, ta.offset + i * G * TWS,
                               [[3 * G * TWS, 128], [TWS, G], [1, TW]])

                    def wb(j, width):
                        return AP(wv.tensor,
                                  wv.offset + i * 384 + g * G * 3 + j,
                                  [[1152, 128], [3, G], [0, width]])

                    # j=0 via affine_select: zero-fills the block (incl.
                    # the pad column) and places the diagonal (c - xi ==
                    # 0) in one gpsimd pass
                    nc.gpsimd.affine_select(
                        out_i, wb(0, TWS),
                        pattern=[[0, G], [1, TWS]], base=0,
                        channel_multiplier=-1,
                        compare_op=mybir.AluOpType.is_equal, fill=0.0,
                    )
                    for j in range(1, 3):
                        ma = masks[:, j, :]
                        mb = AP(ma.tensor, ma.offset,
                                [[3 * TW, 128], [0, G], [1, TW]])
                        nc.vector.copy_predicated(out_w, mb, wb(j, TW))
                    # round to f32r (the BIR verifier requires every
                    # writer of matmul operand bytes to produce f32r, so
                    # the rounded copy lives in its own tile); flat 2D
                    # copy on ACT -- gpsimd tensor_scalar writing f32r
                    # hits a software-emulated store path on HW (~10us
                    # per op, 6x the cost model), so ACT does them all
                    flat_in = AP(ta.tensor, ta.offset + i * G * TWS,
                                 [[3 * G * TWS, 128], [1, G * TWS]])
                    flat_out = AP(ti.tensor, ti.offset + i * G * TWS,
                                  [[3 * G * TWS, 128], [1, G * TWS]])
                    nc.scalar.activation(
                        flat_out, flat_in,
                        mybir.ActivationFunctionType.Copy,
                    )
                tt[g] = t

            rep_range = range(repeat)
            for _rep in rep_range:
              if _rep:
                xt.clear()
                tt.clear()
              # prime the pipeline: fill every buffer slot so the DMA
              # engines saturate from t=0 instead of ramping with the loop
              for _c in range(4):
                  load_chunk(_c)
              for _g in range(5):
                  load_group(_g)

              ystage = None
              for yp in range(H // 2):
                  # prefetch beyond what this row-pair touches
                  load_chunk((2 * yp + 2) // LR + 1)
                  load_chunk((2 * yp + 2) // LR + 2)
                  load_group((2 * yp + 2) // G + 1)
                  load_group((2 * yp + 2) // G + 2)

                  # two output rows share a 2-bank PSUM tile so the
                  # sigmoid/tanh runs once per pair (halves the ACT op
                  # count and its PSUM-access fixed costs); each matmul
                  # still writes a single bank
                  pt = ppool.tile([128, 2 * BC], f32, tag="psum")
                  for half in range(2):
                      yo = 2 * yp + half
                      yis = [yi for yi in (yo - 1, yo, yo + 1)
                             if 0 <= yi < H]
                      for k, yi in enumerate(yis):
                          i_dy = yo - yi + 1
                          lhsT = tt[yi // G][:, i_dy, yi % G, 1 : 1 + 128]
                          rhs = xt[yi // LR][:, yi % LR, :]
                          nc.tensor.matmul(
                              pt[:, half * BC : (half + 1) * BC],
                              lhsT,
                              rhs,
                              start=(k == 0),
                              stop=(k == len(yis) - 1),
                          )

                  yo = 2 * yp
                  if yo % R == 0:
                      ystage = opool.tile([128, R, BC],
                                          f8 if OUT_FP8 else f16, tag="yst")
                  po = ystage[:]
                  pair_out = AP(po.tensor, po.offset + (yo % R) * BC,
                                [[R * BC, 128], [1, 2 * BC]])
                  if OUT_FP8:
                      nc.scalar.activation(
                          pair_out,
                          pt[:],
                          mybir.ActivationFunctionType.Tanh,
                          scale=0.5,
                      )
                  else:
                      nc.scalar.activation(
                          pair_out,
                          pt[:],
                          mybir.ActivationFunctionType.Sigmoid,
                      )
                  if yo % R == R - 2:
                      c = yo // R
                      dst = AP(
                          y.ap().tensor,
                          c * R * 128 * BC,
                          [[BC, 128], [128 * BC, R], [1, BC]],
                      )
                      # stores on the ACT-issued HWDGE ring so they never
                      # head-of-line-block the loads on the SP ring
                      nc.scalar.dma_start(out=dst, in_=ystage[:])
    return nc


def get_nc():
    if "nc" not in _CACHE:
        _CACHE["nc"] = _build_nc()
    return _CACHE["nc"]


def _prep_wsb(w: np.ndarray) -> np.ndarray:
    return np.ascontiguousarray(
        np.asarray(w, dtype=np.float32)
        .reshape(3, 3, H, W)
        .transpose(3, 0, 2, 1)
        .reshape(128, 1152)
    )


def kernel(x: np.ndarray, w: np.ndarray) -> np.ndarray:
    import time as _time

    from concourse.bass_utils import run_bass_kernel_spmd

    x = np.ascontiguousarray(x, dtype=np.float32)
    wsb = _prep_wsb(w)
    nc = get_nc()
    in_maps = [
        {"x": x[:, i * BC : (i + 1) * BC], "wsb": wsb} for i in range(NCORES)
    ]
    # The compile hook / remote execution path occasionally fails
    # transiently (observed: a flaky walrus invocation and a recoverable
    # NRT exec error); retry a few times before giving up.
    last_exc = None
    for attempt in range(4):
        try:
            res = run_bass_kernel_spmd(
                nc, in_maps, list(range(NCORES))
            ).results
            break
        except Exception as exc:  # noqa: BLE001
            last_exc = exc
            _time.sleep(2.0 * (attempt + 1))
    else:
        raise last_exc
    yfull = np.concatenate(
        [np.asarray(res[i]["y"]).astype(np.float32) for i in range(NCORES)],
        axis=1,
    )
    if OUT_FP8:
        # y stored as tanh(z/2): sigmoid(z) = (1 + tanh(z/2)) / 2
        yfull = 0.5 + 0.5 * yfull
    return np.ascontiguousarray(yfull, dtype=np.float32)
